# revision 11
# baseline (speedup 1.0000x reference)
"""Mesa-layer memory kernel for Trainium2 (8 NeuronCores, data-parallel over B).

Math: the reference's T-step Sherman-Morrison / discounted-accumulation
recurrence has a closed form,
    R_final = (I + K^T K)^{-1}            (eps term is O(1e-6) relative)
    S_final^T = K^T diag(c) V,   c_t = prod_{s>t} gamma_s
so per memory b the output is
    out_b = Q_b @ (R_b @ S_b^T).
R is computed with Newton-Schulz iterations (pure matmuls; I + K^T K is
well-conditioned here). Iterations run in bf16 with fp32 refinement at the
end; the big T-contracted matmuls and the query readout run in bf16
(validated ~3e-3 max-rel vs the fp32 reference).
c is computed in log space with a free-dim prefix-sum scan.

Each core owns B/8 = 8 independent memories; no cross-core communication.
"""

import numpy as np

B, T, DK, DV, NQ = 64, 2048, 128, 128, 2048
NCORES = 8
BPC = B // NCORES          # memories per core
P = 128                    # partitions
NCH = T // P               # 16 chunks of 128 timesteps
GCLAMP = 1e-30             # gamma clamp before log (exact-0 gammas)

NS_BF = 6                  # Newton-Schulz iterations in bf16
NS_FP = 2                  # fp32 refinement iterations


def build_nc(ns_bf=NS_BF, ns_fp=NS_FP):
    import concourse.mybir as mybir
    import concourse.tile as tile
    from concourse import bacc
    from concourse.masks import make_identity

    fp32 = mybir.dt.float32
    bf16 = mybir.dt.bfloat16
    AF = mybir.ActivationFunctionType
    OP = mybir.AluOpType
    AX = mybir.AxisListType

    # Bacc (not raw Bass): its compile() pass splits multi-sem sync waits to
    # the 1-wait-per-instruction limit the TRN2 encodings require.
    nc = bacc.Bacc(trn_type="TRN2", target_bir_lowering=False, debug=False)
    keys = nc.dram_tensor("keys", [BPC, T, DK], fp32, kind="ExternalInput").ap()
    values = nc.dram_tensor("values", [BPC, T, DV], fp32, kind="ExternalInput").ap()
    gammas = nc.dram_tensor("gammas", [BPC, T], fp32, kind="ExternalInput").ap()
    queries = nc.dram_tensor("queries", [BPC, NQ, DK], fp32, kind="ExternalInput").ap()
    out = nc.dram_tensor("out", [BPC, NQ, DV], fp32, kind="ExternalOutput").ap()

    with tile.TileContext(nc) as tc:
        const = tc.alloc_tile_pool(name="const", bufs=1)
        gam = tc.alloc_tile_pool(name="gam", bufs=1)
        kp = tc.alloc_tile_pool(name="kp", bufs=2)
        vp = tc.alloc_tile_pool(name="vp", bufs=2)
        kvbp = tc.alloc_tile_pool(name="kvbp", bufs=2)
        qp = tc.alloc_tile_pool(name="qp", bufs=2)
        qtp = tc.alloc_tile_pool(name="qtp", bufs=2)
        small = tc.alloc_tile_pool(name="small", bufs=1)
        xs = tc.alloc_tile_pool(name="xs", bufs=2)
        outp = tc.alloc_tile_pool(name="outp", bufs=2)
        ps_as = tc.alloc_tile_pool(name="ps_as", bufs=2, space="PSUM")
        ps_w = tc.alloc_tile_pool(name="ps_w", bufs=4, space="PSUM")

        ident = const.tile([P, P], fp32)
        make_identity(nc, ident)
        # 4 identity blocks side by side, for group-batched I - A@X residuals
        ident4 = const.tile([P, 4 * P], fp32)
        for i in range(4):
            make_identity(nc, ident4[:, i * P : (i + 1) * P])

        # ---- phase 0: per-memory suffix cumprod of gammas (log space) ----
        g_sb = gam.tile([BPC, T], fp32)
        nc.sync.dma_start(g_sb[:], gammas[:, :])
        c8 = gam.tile([BPC, T], fp32)          # also the scan's zero operand
        nc.vector.memset(c8[:], 0.0)
        nc.vector.tensor_scalar_max(g_sb[:], g_sb[:], GCLAMP)
        nc.scalar.activation(g_sb[:], g_sb[:], AF.Ln)
        incl = gam.tile([BPC, T], fp32)
        # joiner: make DVE observe the ACT (Ln) dependency before the scan
        joiner = gam.tile([BPC, 1], fp32)
        nc.vector.tensor_copy(out=joiner[:], in_=g_sb[:, 0:1])
        nc.vector.tensor_tensor_scan(
            incl[:], g_sb[:], c8[:], 0.0, OP.add, OP.add
        )
        # c = exp(total - incl);  total = inclusive sum at t = T-1
        nc.scalar.activation(
            c8[:], incl[:], AF.Exp, bias=incl[:, T - 1 : T], scale=-1.0
        )
        # PE warmup op: absorb the gpsimd(identity) dependency into PE's clock
        ps_warm = ps_w.tile([BPC, BPC], fp32, tag="w", name="ps_warm")
        nc.tensor.transpose(ps_warm[:], ident[:BPC, :BPC], ident[:BPC, :BPC])
        # transpose to [t-within-chunk (partitions), chunk, memory]
        c_t = gam.tile([P, NCH, BPC], fp32)
        for j in range(NCH):
            ps_ct = ps_w.tile([P, BPC], fp32, tag="w", name=f"ps_ct{j}")
            nc.tensor.transpose(ps_ct[:], c8[:, j * P : (j + 1) * P], ident[:BPC, :BPC])
            nc.vector.tensor_copy(out=c_t[:, j, :], in_=ps_ct[:])

        # ---- per-memory state tiles ----
        A_sb = [small.tile([P, P], fp32, tag=f"A{i}", name=f"A{i}") for i in range(BPC)]
        A_bf = [small.tile([P, P], bf16, tag=f"Ab{i}", name=f"Ab{i}") for i in range(BPC)]
        ST_sb = [small.tile([P, P], fp32, tag=f"S{i}", name=f"S{i}") for i in range(BPC)]
        Phi_bf = [small.tile([P, P], bf16, tag=f"Pb{i}", name=f"Phib{i}") for i in range(BPC)]
        rs_sb = [small.tile([P, 1], fp32, tag=f"r{i}", name=f"rs{i}") for i in range(BPC)]

        # ---- phase 1: load K/V; build bf16 [K | diag(c)V]; A and S^T ----
        for i in range(BPC):
            k_sb = kp.tile([P, NCH, DK], fp32, tag="k", name=f"k{i}")
            nc.sync.dma_start(k_sb[:], keys[i].rearrange("(j p) k -> p j k", p=P))
            v_sb = vp.tile([P, NCH, DV], fp32, tag="v", name=f"v{i}")
            nc.sync.dma_start(v_sb[:], values[i].rearrange("(j p) k -> p j k", p=P))

            kvb = kvbp.tile([P, NCH, 2 * P], bf16, tag="kvb", name=f"kvb{i}")
            # K cast fp32->bf16 on the (otherwise idle) GpSimd engine
            nc.gpsimd.tensor_copy(out=kvb[:, :, 0:DK], in_=k_sb[:])
            # V * c (suffix cumprod weights), broadcast along DV, cast to bf16
            nc.vector.tensor_tensor(
                kvb[:, :, DK : 2 * DK],
                v_sb[:],
                c_t[:, :, i, None].to_broadcast((P, NCH, DV)),
                OP.mult,
            )

            ps = ps_as.tile([P, 2 * P], fp32, tag="as", name=f"ps_as{i}")
            for j in range(NCH):
                nc.tensor.matmul(
                    ps[:],
                    kvb[:, j, 0:DK],
                    kvb[:, j, :],
                    start=(j == 0),
                    stop=(j == NCH - 1),
                )

            # A = K^T K + I ; S^T = K^T diag(c) V ; Jacobi-style NS init
            nc.vector.tensor_tensor(A_sb[i][:], ps[:, 0:P], ident[:], OP.add)
            nc.vector.tensor_copy(out=ST_sb[i][:], in_=ps[:, P : 2 * P])
            nc.vector.tensor_copy(out=A_bf[i][:], in_=A_sb[i][:])
            nc.vector.tensor_reduce(
                rs_sb[i][:], A_sb[i][:], AX.X, OP.add, apply_absolute_value=True
            )
            nc.vector.reciprocal(rs_sb[i][:], rs_sb[i][:])

        # ---- phase 2: Newton-Schulz, batched 4 memories per PSUM bank ----
        NG = BPC // 4  # groups of 4 memories
        Xb = []
        for g in range(NG):
            xw = xs.tile([P, 4 * P], bf16, tag=f"Xb{g}", name=f"Xb{g}_0")
            for i in range(4):
                m = 4 * g + i
                nc.scalar.activation(
                    xw[:, i * P : (i + 1) * P], ident[:], AF.Copy,
                    scale=rs_sb[m][:],
                )
            Xb.append(xw)

        # Residual form X <- X + X^T (I - A X): the asymmetry that bf16
        # storage induces in X enters only multiplied by the residual, so the
        # iteration converges (plain 2X - X^T A X stalls at ~kappa*bf16eps).
        for it in range(ns_bf + ns_fp):
            bf_iter = it < ns_bf
            last_bf = it == ns_bf - 1
            Amat = A_bf if bf_iter else A_sb
            ps_a = []
            for g in range(NG):
                pa = ps_w.tile([P, 4 * P], fp32, tag="w", name=f"pa{it}_{g}")
                for i in range(4):
                    sl = slice(i * P, (i + 1) * P)
                    nc.tensor.matmul(pa[:, sl], Amat[4 * g + i][:], Xb[g][:, sl])
                ps_a.append(pa)
            ex = []
            for g in range(NG):
                # E = I - A X
                eg = xs.tile(
                    [P, 4 * P], bf16 if bf_iter else fp32,
                    tag=f"e{g}_{bf_iter}", name=f"e{it}_{g}",
                )
                nc.vector.scalar_tensor_tensor(
                    eg[:], ps_a[g][:], -1.0, ident4[:], OP.mult, OP.add
                )
                ex.append(eg)
            ps_b = []
            for g in range(NG):
                pb = ps_w.tile([P, 4 * P], fp32, tag="w", name=f"pb{it}_{g}")
                for i in range(4):
                    sl = slice(i * P, (i + 1) * P)
                    nc.tensor.matmul(pb[:, sl], Xb[g][:, sl], ex[g][:, sl])
                ps_b.append(pb)
            for g in range(NG):
                out_fp32 = (not bf_iter) or last_bf
                xn = xs.tile(
                    [P, 4 * P], fp32 if out_fp32 else bf16,
                    tag=f"Xf{g}" if out_fp32 else f"Xb{g}",
                    name=f"X{g}_{it + 1}",
                )
                nc.vector.tensor_tensor(xn[:], Xb[g][:], ps_b[g][:], OP.add)
                Xb[g] = xn

        # ---- phase 3: Phi = R @ S^T (fp32 matmul, bf16 result) ----
        for i in range(BPC):
            g, sl = i // 4, slice((i % 4) * P, (i % 4 + 1) * P)
            ps_phi = ps_w.tile([P, P], fp32, tag="w", name=f"ps_phi{i}")
            nc.tensor.matmul(ps_phi[:], Xb[g][:, sl], ST_sb[i][:])
            nc.vector.tensor_copy(out=Phi_bf[i][:], in_=ps_phi[:])

        # ---- phase 4: out = Q @ Phi; transposes and matmuls packed 4/bank ----
        for i in range(BPC):
            q_sb = qp.tile([P, NCH, DK], fp32, tag="q", name=f"q{i}")
            nc.sync.dma_start(q_sb[:], queries[i].rearrange("(j p) k -> p j k", p=P))
            qt = qtp.tile([P, NCH, P], bf16, tag="qt", name=f"qt{i}")
            for j4 in range(NCH // 4):
                ps_qt = ps_w.tile([P, 4 * P], fp32, tag="w", name=f"ps_qt{i}_{j4}")
                for j in range(4):
                    nc.tensor.transpose(
                        ps_qt[:, j * P : (j + 1) * P], q_sb[:, 4 * j4 + j, :], ident[:]
                    )
                nc.vector.tensor_copy(
                    out=qt[:, 4 * j4 : 4 * j4 + 4, :], in_=ps_qt[:]
                )
            o_sb = outp.tile([P, NCH, DV], fp32, tag="o", name=f"o{i}")
            for j4 in range(NCH // 4):
                ps_o = ps_w.tile([P, 4 * P], fp32, tag="w", name=f"ps_o{i}_{j4}")
                for j in range(4):
                    nc.tensor.matmul(
                        ps_o[:, j * P : (j + 1) * P], qt[:, 4 * j4 + j, :], Phi_bf[i][:]
                    )
                nc.vector.tensor_copy(
                    out=o_sb[:, 4 * j4 : 4 * j4 + 4, :], in_=ps_o[:]
                )
            nc.sync.dma_start(out[i].rearrange("(j p) v -> p j v", p=P), o_sb[:])

        for pool in (ps_w, ps_as, outp, xs, small, qtp, qp, kvbp, vp, kp, gam, const):
            pool.release()

    if not nc.is_finalized():
        nc.finalize()
    return nc


def kernel(**inputs) -> np.ndarray:
    keys = np.ascontiguousarray(inputs["keys"], dtype=np.float32)
    values = np.ascontiguousarray(inputs["values"], dtype=np.float32)
    gammas = np.ascontiguousarray(inputs["gammas"], dtype=np.float32)
    queries = np.ascontiguousarray(inputs["queries"], dtype=np.float32)

    from concourse.bass_utils import run_bass_kernel_spmd

    nc = build_nc()
    in_maps = []
    for m in range(NCORES):
        s = slice(m * BPC, (m + 1) * BPC)
        in_maps.append(
            {
                "keys": keys[s],
                "values": values[s],
                "gammas": gammas[s],
                "queries": queries[s],
            }
        )
    res = run_bass_kernel_spmd(nc, in_maps, core_ids=list(range(NCORES)))
    return np.concatenate([res.results[m]["out"] for m in range(NCORES)], axis=0)


# revision 16
# speedup vs baseline: 1.0845x; 1.0845x over previous
"""Mesa-layer memory kernel for Trainium2 (8 NeuronCores, data-parallel over B).

Math: the reference's T-step Sherman-Morrison / discounted-accumulation
recurrence has a closed form,
    R_final = (I + K^T K)^{-1}            (eps term is O(1e-6) relative)
    S_final^T = K^T diag(c) V,   c_t = prod_{s>t} gamma_s
so per memory b the output is
    out_b = Q_b @ (R_b @ S_b^T).
R is computed with Newton-Schulz iterations in residual form
    X <- X + X^T (I - A X)
(bf16 iterations + fp32 refinements; the residual form keeps the bf16
asymmetry of X out of the error floor). A and S^T are fp32 matmuls; the
query readout runs in bf16 (validated ~2.7e-3 max-rel vs fp32 reference).
c is computed in log space with a free-dim prefix-sum scan.

Layout trick: timestep t maps to (partition p, slot r) via t = 16 p + r.
The contraction over t only requires K/V (and Q/out for the readout side)
to agree on the partition assignment, and this one makes every DMA a fully
contiguous 1 MB transfer (8 KB per partition), which the DMA engines run
near line rate, instead of 512 B strided pieces.

Each core owns B/8 = 8 independent memories; no cross-core communication.
"""

import numpy as np

B, T, DK, DV, NQ = 64, 2048, 128, 128, 2048
NCORES = 8
BPC = B // NCORES          # memories per core
P = 128                    # partitions
R16 = T // P               # 16 row-slots per partition
GCLAMP = 1e-30             # gamma clamp before log (exact-0 gammas)

NS_BF = 6                  # Newton-Schulz iterations in bf16
NS_FP = 2                  # fp32 refinement iterations


def build_nc(ns_bf=NS_BF, ns_fp=NS_FP):
    import concourse.mybir as mybir
    import concourse.tile as tile
    from concourse import bacc
    from concourse.masks import make_identity

    fp32 = mybir.dt.float32
    bf16 = mybir.dt.bfloat16
    AF = mybir.ActivationFunctionType
    OP = mybir.AluOpType
    AX = mybir.AxisListType

    # Bacc (not raw Bass): its compile() pass splits multi-sem sync waits to
    # the 1-wait-per-instruction limit the TRN2 encodings require.
    nc = bacc.Bacc(trn_type="TRN2", target_bir_lowering=False, debug=False)
    keys = nc.dram_tensor("keys", [BPC, T, DK], fp32, kind="ExternalInput").ap()
    values = nc.dram_tensor("values", [BPC, T, DV], fp32, kind="ExternalInput").ap()
    gammas = nc.dram_tensor("gammas", [BPC, T], fp32, kind="ExternalInput").ap()
    queries = nc.dram_tensor("queries", [BPC, NQ, DK], fp32, kind="ExternalInput").ap()
    out = nc.dram_tensor("out", [BPC, NQ, DV], fp32, kind="ExternalOutput").ap()

    with tile.TileContext(nc) as tc:
        const = tc.alloc_tile_pool(name="const", bufs=1)
        gam = tc.alloc_tile_pool(name="gam", bufs=1)
        kp = tc.alloc_tile_pool(name="kp", bufs=2)
        vp = tc.alloc_tile_pool(name="vp", bufs=2)
        vcp = tc.alloc_tile_pool(name="vcp", bufs=2)
        qp = tc.alloc_tile_pool(name="qp", bufs=2)
        qbp = tc.alloc_tile_pool(name="qbp", bufs=2)
        qtp = tc.alloc_tile_pool(name="qtp", bufs=2)
        small = tc.alloc_tile_pool(name="small", bufs=1)
        xs = tc.alloc_tile_pool(name="xs", bufs=2)
        outp = tc.alloc_tile_pool(name="outp", bufs=2)
        ps_as = tc.alloc_tile_pool(name="ps_as", bufs=2, space="PSUM")
        ps_w = tc.alloc_tile_pool(name="ps_w", bufs=4, space="PSUM")

        ident = const.tile([P, P], fp32)
        make_identity(nc, ident)
        ident_bf = const.tile([P, P], bf16)
        make_identity(nc, ident_bf)
        # 4 identity blocks side by side, for group-batched I - A@X residuals
        ident4 = const.tile([P, 4 * P], fp32)
        for i in range(4):
            make_identity(nc, ident4[:, i * P : (i + 1) * P])

        # ---- phase 0: per-memory suffix cumprod of gammas (log space) ----
        g_sb = gam.tile([BPC, T], fp32)
        nc.sync.dma_start(g_sb[:], gammas[:, :])
        c8 = gam.tile([BPC, T], fp32)          # also the scan's zero operand
        nc.vector.memset(c8[:], 0.0)
        nc.vector.tensor_scalar_max(g_sb[:], g_sb[:], GCLAMP)
        nc.scalar.activation(g_sb[:], g_sb[:], AF.Ln)
        incl = gam.tile([BPC, T], fp32)
        # joiner: make DVE observe the ACT (Ln) dependency before the scan
        joiner = gam.tile([BPC, 1], fp32)
        nc.vector.tensor_copy(out=joiner[:], in_=g_sb[:, 0:1])
        nc.vector.tensor_tensor_scan(
            incl[:], g_sb[:], c8[:], 0.0, OP.add, OP.add
        )
        # c = exp(total - incl);  total = inclusive sum at t = T-1
        nc.scalar.activation(
            c8[:], incl[:], AF.Exp, bias=incl[:, T - 1 : T], scale=-1.0
        )
        # PE warmup op: absorb the gpsimd(identity) dependency into PE's clock
        ps_warm = ps_w.tile([BPC, BPC], fp32, tag="w", name="ps_warm")
        nc.tensor.transpose(ps_warm[:], ident[:BPC, :BPC], ident[:BPC, :BPC])
        # c_t[p, r, i] = c8[i, 16p + r]: strided PE transposes
        c_t = gam.tile([P, R16, BPC], fp32)
        c8_r = c8.rearrange("i (p r) -> i r p", r=R16)
        for r in range(R16):
            ps_ct = ps_w.tile([P, BPC], fp32, tag="w", name=f"ps_ct{r}")
            nc.tensor.transpose(ps_ct[:], c8_r[:, r, :], ident[:BPC, :BPC])
            nc.vector.tensor_copy(out=c_t[:, r, :], in_=ps_ct[:])

        # ---- per-memory state tiles ----
        A_sb = [small.tile([P, P], fp32, tag=f"A{i}", name=f"A{i}") for i in range(BPC)]
        A_bf = [small.tile([P, P], bf16, tag=f"Ab{i}", name=f"Ab{i}") for i in range(BPC)]
        ST_sb = [small.tile([P, P], fp32, tag=f"S{i}", name=f"S{i}") for i in range(BPC)]
        Phi_bf = [small.tile([P, P], bf16, tag=f"Pb{i}", name=f"Phib{i}") for i in range(BPC)]
        rs_sb = [small.tile([P, 1], fp32, tag=f"r{i}", name=f"rs{i}") for i in range(BPC)]

        # ---- phase 1: load K/V (contiguous); A = K^T K + I, S^T = K^T (cV) ----
        for i in range(BPC):
            k_sb = kp.tile([P, R16, DK], fp32, tag="k", name=f"k{i}")
            nc.sync.dma_start(k_sb[:], keys[i].rearrange("(p r) k -> p r k", p=P))
            v_sb = vp.tile([P, R16, DV], fp32, tag="v", name=f"v{i}")
            nc.sync.dma_start(v_sb[:], values[i].rearrange("(p r) k -> p r k", p=P))

            # V * c (suffix cumprod weights), broadcast along DV, fp32
            vc = vcp.tile([P, R16, DV], fp32, tag="vc", name=f"vc{i}")
            nc.vector.tensor_tensor(
                vc[:], v_sb[:],
                c_t[:, :, i, None].to_broadcast((P, R16, DV)),
                OP.mult,
            )

            ps_a = ps_as.tile([P, P], fp32, tag="psa", name=f"ps_a{i}")
            ps_s = ps_as.tile([P, P], fp32, tag="pss", name=f"ps_s{i}")
            for r in range(R16):
                nc.tensor.matmul(
                    ps_a[:], k_sb[:, r, :], k_sb[:, r, :],
                    start=(r == 0), stop=(r == R16 - 1),
                )
                nc.tensor.matmul(
                    ps_s[:], k_sb[:, r, :], vc[:, r, :],
                    start=(r == 0), stop=(r == R16 - 1),
                )

            # A = K^T K + I ; S^T = K^T diag(c) V ; Jacobi-style NS init
            nc.vector.tensor_tensor(A_sb[i][:], ps_a[:], ident[:], OP.add)
            nc.vector.tensor_copy(out=ST_sb[i][:], in_=ps_s[:])
            nc.vector.tensor_copy(out=A_bf[i][:], in_=A_sb[i][:])
            nc.vector.tensor_reduce(
                rs_sb[i][:], A_sb[i][:], AX.X, OP.add, apply_absolute_value=True
            )
            nc.vector.reciprocal(rs_sb[i][:], rs_sb[i][:])

        # ---- phase 2: Newton-Schulz (residual form), 4 memories per bank ----
        NG = BPC // 4
        Xb = []
        for g in range(NG):
            xw = xs.tile([P, 4 * P], bf16, tag=f"Xb{g}", name=f"Xb{g}_0")
            for i in range(4):
                nc.scalar.activation(
                    xw[:, i * P : (i + 1) * P], ident[:], AF.Copy,
                    scale=rs_sb[4 * g + i][:],
                )
            Xb.append(xw)

        for it in range(ns_bf + ns_fp):
            bf_iter = it < ns_bf
            last_bf = it == ns_bf - 1
            Amat = A_bf if bf_iter else A_sb
            ps_l = []
            for g in range(NG):
                pa = ps_w.tile([P, 4 * P], fp32, tag="w", name=f"pa{it}_{g}")
                for i in range(4):
                    sl = slice(i * P, (i + 1) * P)
                    nc.tensor.matmul(pa[:, sl], Amat[4 * g + i][:], Xb[g][:, sl])
                ps_l.append(pa)
            ex = []
            for g in range(NG):
                eg = xs.tile(
                    [P, 4 * P], bf16 if bf_iter else fp32,
                    tag=f"e{g}_{bf_iter}", name=f"e{it}_{g}",
                )
                nc.vector.scalar_tensor_tensor(
                    eg[:], ps_l[g][:], -1.0, ident4[:], OP.mult, OP.add
                )
                ex.append(eg)
            ps_b = []
            for g in range(NG):
                pb = ps_w.tile([P, 4 * P], fp32, tag="w", name=f"pb{it}_{g}")
                for i in range(4):
                    sl = slice(i * P, (i + 1) * P)
                    nc.tensor.matmul(pb[:, sl], Xb[g][:, sl], ex[g][:, sl])
                ps_b.append(pb)
            for g in range(NG):
                out_fp32 = (not bf_iter) or last_bf
                xn = xs.tile(
                    [P, 4 * P], fp32 if out_fp32 else bf16,
                    tag=f"Xf{g}" if out_fp32 else f"Xb{g}",
                    name=f"X{g}_{it + 1}",
                )
                nc.vector.tensor_tensor(xn[:], Xb[g][:], ps_b[g][:], OP.add)
                Xb[g] = xn

        # ---- phase 3: Phi = R @ S^T (fp32 matmul, bf16 result) ----
        for i in range(BPC):
            g, sl = i // 4, slice((i % 4) * P, (i % 4 + 1) * P)
            ps_phi = ps_w.tile([P, P], fp32, tag="w", name=f"ps_phi{i}")
            nc.tensor.matmul(ps_phi[:], Xb[g][:, sl], ST_sb[i][:])
            nc.vector.tensor_copy(out=Phi_bf[i][:], in_=ps_phi[:])

        # ---- phase 4: out = Q @ Phi (bf16 readout, packed 4 per bank) ----
        for i in range(BPC):
            q_sb = qp.tile([P, R16, DK], fp32, tag="q", name=f"q{i}")
            nc.scalar.dma_start(q_sb[:], queries[i].rearrange("(p r) k -> p r k", p=P))
            # one big fp32->bf16 cast on the otherwise idle GpSimd engine
            q_bf = qbp.tile([P, R16, DK], bf16, tag="qb", name=f"qb{i}")
            nc.gpsimd.tensor_copy(out=q_bf[:], in_=q_sb[:])
            qt = qtp.tile([P, R16, P], bf16, tag="qt", name=f"qt{i}")
            for r4 in range(R16 // 4):
                ps_qt = ps_w.tile([P, 4 * P], bf16, tag="w", name=f"ps_qt{i}_{r4}")
                for j in range(4):
                    nc.tensor.transpose(
                        ps_qt[:, j * P : (j + 1) * P], q_bf[:, 4 * r4 + j, :],
                        ident_bf[:],
                    )
                nc.vector.tensor_copy(
                    out=qt[:, 4 * r4 : 4 * r4 + 4, :], in_=ps_qt[:]
                )
            o_sb = outp.tile([P, R16, DV], fp32, tag="o", name=f"o{i}")
            for r4 in range(R16 // 4):
                ps_o = ps_w.tile([P, 4 * P], fp32, tag="w", name=f"ps_o{i}_{r4}")
                for j in range(4):
                    nc.tensor.matmul(
                        ps_o[:, j * P : (j + 1) * P], qt[:, 4 * r4 + j, :],
                        Phi_bf[i][:],
                    )
                nc.vector.tensor_copy(
                    out=o_sb[:, 4 * r4 : 4 * r4 + 4, :], in_=ps_o[:]
                )
            nc.scalar.dma_start(out[i].rearrange("(p r) v -> p r v", p=P), o_sb[:])

        for pool in (ps_w, ps_as, outp, xs, small, qtp, qbp, qp, vcp, vp, kp,
                     gam, const):
            pool.release()

    if not nc.is_finalized():
        nc.finalize()
    return nc


def kernel(**inputs) -> np.ndarray:
    keys = np.ascontiguousarray(inputs["keys"], dtype=np.float32)
    values = np.ascontiguousarray(inputs["values"], dtype=np.float32)
    gammas = np.ascontiguousarray(inputs["gammas"], dtype=np.float32)
    queries = np.ascontiguousarray(inputs["queries"], dtype=np.float32)

    from concourse.bass_utils import run_bass_kernel_spmd

    nc = build_nc()
    in_maps = []
    for m in range(NCORES):
        s = slice(m * BPC, (m + 1) * BPC)
        in_maps.append(
            {
                "keys": keys[s],
                "values": values[s],
                "gammas": gammas[s],
                "queries": queries[s],
            }
        )
    res = run_bass_kernel_spmd(nc, in_maps, core_ids=list(range(NCORES)))
    return np.concatenate([res.results[m]["out"] for m in range(NCORES)], axis=0)


# revision 17
# speedup vs baseline: 1.1875x; 1.0950x over previous
"""Mesa-layer memory kernel for Trainium2 (8 NeuronCores, data-parallel over B).

Math: the reference's T-step Sherman-Morrison / discounted-accumulation
recurrence has a closed form,
    R_final = (I + K^T K)^{-1}            (eps term is O(1e-6) relative)
    S_final^T = K^T diag(c) V,   c_t = prod_{s>t} gamma_s
so per memory b the output is
    out_b = Q_b @ (R_b @ S_b^T).
R is computed with Newton-Schulz iterations in residual form
    X <- X + X^T (I - A X)
(bf16 iterations + fp32 refinements; the residual form keeps the bf16
asymmetry of X out of the error floor). The T-contracted matmuls and the
query readout run in bf16 (validated ~3.3e-3 max-rel vs fp32 reference);
fp32->bf16 casts are spread across the otherwise-idle Scalar and GpSimd
engines.
c is computed in log space with a free-dim prefix-sum scan.

Layout trick: timestep t maps to (partition p, slot r) via t = 16 p + r.
The contraction over t only requires K/V (and Q/out for the readout side)
to agree on the partition assignment, and this one makes every DMA a fully
contiguous 1 MB transfer (8 KB per partition), which the DMA engines run
near line rate, instead of 512 B strided pieces.

Each core owns B/8 = 8 independent memories; no cross-core communication.
"""

import numpy as np

B, T, DK, DV, NQ = 64, 2048, 128, 128, 2048
NCORES = 8
BPC = B // NCORES          # memories per core
P = 128                    # partitions
R16 = T // P               # 16 row-slots per partition
GCLAMP = 1e-30             # gamma clamp before log (exact-0 gammas)

NS_BF = 6                  # Newton-Schulz iterations in bf16
NS_FP = 2                  # fp32 refinement iterations


def build_nc(ns_bf=NS_BF, ns_fp=NS_FP):
    import concourse.mybir as mybir
    import concourse.tile as tile
    from concourse import bacc
    from concourse.masks import make_identity

    fp32 = mybir.dt.float32
    bf16 = mybir.dt.bfloat16
    AF = mybir.ActivationFunctionType
    OP = mybir.AluOpType
    AX = mybir.AxisListType

    # Bacc (not raw Bass): its compile() pass splits multi-sem sync waits to
    # the 1-wait-per-instruction limit the TRN2 encodings require.
    nc = bacc.Bacc(trn_type="TRN2", target_bir_lowering=False, debug=False)
    keys = nc.dram_tensor("keys", [BPC, T, DK], fp32, kind="ExternalInput").ap()
    values = nc.dram_tensor("values", [BPC, T, DV], fp32, kind="ExternalInput").ap()
    gammas = nc.dram_tensor("gammas", [BPC, T], fp32, kind="ExternalInput").ap()
    queries = nc.dram_tensor("queries", [BPC, NQ, DK], fp32, kind="ExternalInput").ap()
    out = nc.dram_tensor("out", [BPC, NQ, DV], fp32, kind="ExternalOutput").ap()

    with tile.TileContext(nc) as tc:
        const = tc.alloc_tile_pool(name="const", bufs=1)
        gam = tc.alloc_tile_pool(name="gam", bufs=1)
        kp = tc.alloc_tile_pool(name="kp", bufs=2)
        vp = tc.alloc_tile_pool(name="vp", bufs=2)
        kvbp = tc.alloc_tile_pool(name="kvbp", bufs=2)
        qp = tc.alloc_tile_pool(name="qp", bufs=2)
        qbp = tc.alloc_tile_pool(name="qbp", bufs=1)
        qtp = tc.alloc_tile_pool(name="qtp", bufs=2)
        small = tc.alloc_tile_pool(name="small", bufs=1)
        xs = tc.alloc_tile_pool(name="xs", bufs=2)
        outp = tc.alloc_tile_pool(name="outp", bufs=2)
        ps_as = tc.alloc_tile_pool(name="ps_as", bufs=2, space="PSUM")
        ps_w = tc.alloc_tile_pool(name="ps_w", bufs=5, space="PSUM")

        ident = const.tile([P, P], fp32)
        make_identity(nc, ident)
        ident_bf = const.tile([P, P], bf16)
        make_identity(nc, ident_bf)
        # 4 identity blocks side by side, for group-batched I - A@X residuals
        ident4 = const.tile([P, 4 * P], fp32)
        for i in range(4):
            make_identity(nc, ident4[:, i * P : (i + 1) * P])

        # ---- phase 0: per-memory suffix cumprod of gammas (log space) ----
        g_sb = gam.tile([BPC, T], fp32)
        nc.sync.dma_start(g_sb[:], gammas[:, :])
        c8 = gam.tile([BPC, T], fp32)          # also the scan's zero operand
        nc.vector.memset(c8[:], 0.0)
        nc.vector.tensor_scalar_max(g_sb[:], g_sb[:], GCLAMP)
        nc.scalar.activation(g_sb[:], g_sb[:], AF.Ln)
        incl = gam.tile([BPC, T], fp32)
        # joiner: make DVE observe the ACT (Ln) dependency before the scan
        joiner = gam.tile([BPC, 1], fp32)
        nc.vector.tensor_copy(out=joiner[:], in_=g_sb[:, 0:1])
        nc.vector.tensor_tensor_scan(
            incl[:], g_sb[:], c8[:], 0.0, OP.add, OP.add
        )
        # c = exp(total - incl);  total = inclusive sum at t = T-1
        nc.scalar.activation(
            c8[:], incl[:], AF.Exp, bias=incl[:, T - 1 : T], scale=-1.0
        )
        # PE warmup op: absorb the gpsimd(identity) dependency into PE's clock
        ps_warm = ps_w.tile([BPC, BPC], fp32, tag="w", name="ps_warm")
        nc.tensor.transpose(ps_warm[:], ident[:BPC, :BPC], ident[:BPC, :BPC])
        # c_t[p, r, i] = c8[i, 16p + r]: strided PE transposes
        c_t = gam.tile([P, R16, BPC], fp32)
        c8_r = c8.rearrange("i (p r) -> i r p", r=R16)
        for r in range(R16):
            ps_ct = ps_w.tile([P, BPC], fp32, tag="w", name=f"ps_ct{r}")
            nc.tensor.transpose(ps_ct[:], c8_r[:, r, :], ident[:BPC, :BPC])
            nc.vector.tensor_copy(out=c_t[:, r, :], in_=ps_ct[:])

        # ---- per-memory state tiles ----
        A_sb = [small.tile([P, P], fp32, tag=f"A{i}", name=f"A{i}") for i in range(BPC)]
        A_bf = [small.tile([P, P], bf16, tag=f"Ab{i}", name=f"Ab{i}") for i in range(BPC)]
        ST_sb = [small.tile([P, P], fp32, tag=f"S{i}", name=f"S{i}") for i in range(BPC)]
        Phi_bf = [small.tile([P, P], bf16, tag=f"Pb{i}", name=f"Phib{i}") for i in range(BPC)]
        rs_sb = [small.tile([P, 1], fp32, tag=f"r{i}", name=f"rs{i}") for i in range(BPC)]
        Qb = [qbp.tile([P, R16, DK], bf16, tag=f"qb{i}", name=f"qb{i}") for i in range(BPC)]

        # ---- phase 1: load K/V/Q; bf16 [K | cV]; A = K^T K + I, S^T ----
        for i in range(BPC):
            k_sb = kp.tile([P, R16, DK], fp32, tag="k", name=f"k{i}")
            nc.sync.dma_start(k_sb[:], keys[i].rearrange("(p r) k -> p r k", p=P))
            v_sb = vp.tile([P, R16, DV], fp32, tag="v", name=f"v{i}")
            nc.sync.dma_start(v_sb[:], values[i].rearrange("(p r) k -> p r k", p=P))
            q_sb = qp.tile([P, R16, DK], fp32, tag="q", name=f"q{i}")
            nc.scalar.dma_start(q_sb[:], queries[i].rearrange("(p r) k -> p r k", p=P))

            kvb = kvbp.tile([P, R16, 2 * P], bf16, tag="kvb", name=f"kvb{i}")
            # casts split across the two otherwise-idle engines, alternating
            if i % 2 == 0:
                nc.scalar.copy(out=kvb[:, :, 0:DK], in_=k_sb[:])
                nc.gpsimd.tensor_copy(out=Qb[i][:], in_=q_sb[:])
            else:
                nc.gpsimd.tensor_copy(out=kvb[:, :, 0:DK], in_=k_sb[:])
                nc.scalar.copy(out=Qb[i][:], in_=q_sb[:])
            # V * c (suffix cumprod weights), broadcast along DV, bf16 out
            nc.vector.tensor_tensor(
                kvb[:, :, DK : 2 * DK], v_sb[:],
                c_t[:, :, i, None].to_broadcast((P, R16, DV)),
                OP.mult,
            )

            ps = ps_as.tile([P, 2 * P], fp32, tag="as", name=f"ps_as{i}")
            for r in range(R16):
                nc.tensor.matmul(
                    ps[:], kvb[:, r, 0:DK], kvb[:, r, :],
                    start=(r == 0), stop=(r == R16 - 1),
                )

            # A = K^T K + I ; S^T = K^T diag(c) V ; Jacobi-style NS init
            nc.vector.tensor_tensor(A_sb[i][:], ps[:, 0:P], ident[:], OP.add)
            nc.vector.tensor_copy(out=ST_sb[i][:], in_=ps[:, P : 2 * P])
            nc.vector.tensor_copy(out=A_bf[i][:], in_=A_sb[i][:])
            nc.vector.tensor_reduce(
                rs_sb[i][:], A_sb[i][:], AX.X, OP.add, apply_absolute_value=True
            )
            nc.vector.reciprocal(rs_sb[i][:], rs_sb[i][:])

        # ---- phase 2: Newton-Schulz (residual form), 4 memories per bank ----
        NG = BPC // 4
        Xb = []
        for g in range(NG):
            xw = xs.tile([P, 4 * P], bf16, tag=f"Xb{g}", name=f"Xb{g}_0")
            for i in range(4):
                nc.scalar.activation(
                    xw[:, i * P : (i + 1) * P], ident[:], AF.Copy,
                    scale=rs_sb[4 * g + i][:],
                )
            Xb.append(xw)

        for it in range(ns_bf + ns_fp):
            bf_iter = it < ns_bf
            last_bf = it == ns_bf - 1
            Amat = A_bf if bf_iter else A_sb
            ps_l = []
            for g in range(NG):
                pa = ps_w.tile([P, 4 * P], fp32, tag="w", name=f"pa{it}_{g}")
                for i in range(4):
                    sl = slice(i * P, (i + 1) * P)
                    nc.tensor.matmul(pa[:, sl], Amat[4 * g + i][:], Xb[g][:, sl])
                ps_l.append(pa)
            ex = []
            for g in range(NG):
                eg = xs.tile(
                    [P, 4 * P], bf16 if bf_iter else fp32,
                    tag=f"e{g}_{bf_iter}", name=f"e{it}_{g}",
                )
                nc.vector.scalar_tensor_tensor(
                    eg[:], ps_l[g][:], -1.0, ident4[:], OP.mult, OP.add
                )
                ex.append(eg)
            ps_b = []
            for g in range(NG):
                pb = ps_w.tile([P, 4 * P], fp32, tag="w", name=f"pb{it}_{g}")
                for i in range(4):
                    sl = slice(i * P, (i + 1) * P)
                    nc.tensor.matmul(pb[:, sl], Xb[g][:, sl], ex[g][:, sl])
                ps_b.append(pb)
            for g in range(NG):
                out_fp32 = (not bf_iter) or last_bf
                xn = xs.tile(
                    [P, 4 * P], fp32 if out_fp32 else bf16,
                    tag=f"Xf{g}" if out_fp32 else f"Xb{g}",
                    name=f"X{g}_{it + 1}",
                )
                nc.vector.tensor_tensor(xn[:], Xb[g][:], ps_b[g][:], OP.add)
                Xb[g] = xn

        # ---- phase 3: Phi = R @ S^T (fp32 matmul, bf16 result) ----
        for i in range(BPC):
            g, sl = i // 4, slice((i % 4) * P, (i % 4 + 1) * P)
            ps_phi = ps_w.tile([P, P], fp32, tag="w", name=f"ps_phi{i}")
            nc.tensor.matmul(ps_phi[:], Xb[g][:, sl], ST_sb[i][:])
            nc.vector.tensor_copy(out=Phi_bf[i][:], in_=ps_phi[:])

        # ---- phase 4: out = Q @ Phi (bf16 readout, packed 4 per bank) ----
        for i in range(BPC):
            qt = qtp.tile([P, R16, P], bf16, tag="qt", name=f"qt{i}")
            for r4 in range(R16 // 4):
                ps_qt = ps_w.tile([P, 4 * P], bf16, tag="w", name=f"ps_qt{i}_{r4}")
                for j in range(4):
                    nc.tensor.transpose(
                        ps_qt[:, j * P : (j + 1) * P], Qb[i][:, 4 * r4 + j, :],
                        ident_bf[:],
                    )
                nc.vector.tensor_copy(
                    out=qt[:, 4 * r4 : 4 * r4 + 4, :], in_=ps_qt[:]
                )
            o_sb = outp.tile([P, R16, DV], fp32, tag="o", name=f"o{i}")
            for r4 in range(R16 // 4):
                ps_o = ps_w.tile([P, 4 * P], fp32, tag="w", name=f"ps_o{i}_{r4}")
                for j in range(4):
                    nc.tensor.matmul(
                        ps_o[:, j * P : (j + 1) * P], qt[:, 4 * r4 + j, :],
                        Phi_bf[i][:],
                    )
                nc.vector.tensor_copy(
                    out=o_sb[:, 4 * r4 : 4 * r4 + 4, :], in_=ps_o[:]
                )
            nc.scalar.dma_start(out[i].rearrange("(p r) v -> p r v", p=P), o_sb[:])

        for pool in (ps_w, ps_as, outp, xs, small, qtp, qbp, qp, kvbp, vp, kp,
                     gam, const):
            pool.release()

    if not nc.is_finalized():
        nc.finalize()
    return nc


def kernel(**inputs) -> np.ndarray:
    keys = np.ascontiguousarray(inputs["keys"], dtype=np.float32)
    values = np.ascontiguousarray(inputs["values"], dtype=np.float32)
    gammas = np.ascontiguousarray(inputs["gammas"], dtype=np.float32)
    queries = np.ascontiguousarray(inputs["queries"], dtype=np.float32)

    from concourse.bass_utils import run_bass_kernel_spmd

    nc = build_nc()
    in_maps = []
    for m in range(NCORES):
        s = slice(m * BPC, (m + 1) * BPC)
        in_maps.append(
            {
                "keys": keys[s],
                "values": values[s],
                "gammas": gammas[s],
                "queries": queries[s],
            }
        )
    res = run_bass_kernel_spmd(nc, in_maps, core_ids=list(range(NCORES)))
    return np.concatenate([res.results[m]["out"] for m in range(NCORES)], axis=0)


# revision 19
# speedup vs baseline: 1.3533x; 1.1396x over previous
"""Mesa-layer memory kernel for Trainium2 (8 NeuronCores, data-parallel over B).

Math: the reference's T-step Sherman-Morrison / discounted-accumulation
recurrence has a closed form,
    R_final = (I + K^T K)^{-1}            (eps term is O(1e-6) relative)
    S_final^T = K^T diag(c) V,   c_t = prod_{s>t} gamma_s
so per memory b the output is
    out_b = Q_b @ (R_b @ S_b^T).
R is computed with Newton-Schulz iterations in residual form
    X <- X + X^T (I - A X)
(bf16 iterations + fp32 refinements; the residual form keeps the bf16
asymmetry of X out of the error floor). The T-contracted matmuls and the
query readout run in bf16 (validated ~3.3e-3 max-rel vs fp32 reference);
fp32->bf16 casts are spread across the otherwise-idle Scalar and GpSimd
engines.

Layout trick: timestep t maps to (partition p, slot r) via t = 16 p + r.
The contraction over t only requires K/V (and Q/out for the readout side)
to agree on the partition assignment, and this one makes every DMA a fully
contiguous 1 MB transfer (8 KB per partition), which the DMA engines run
near line rate, instead of 512 B strided pieces.

The suffix cumprod of gammas runs in log space: 16-step free-dim scans per
partition + one triangular matmul on the TensorEngine for the
cross-partition prefix (a 2048-step serial scan would cost ~10 us).

Each core owns B/8 = 8 independent memories; no cross-core communication.
"""

import numpy as np

B, T, DK, DV, NQ = 64, 2048, 128, 128, 2048
NCORES = 8
BPC = B // NCORES          # memories per core
P = 128                    # partitions
R16 = T // P               # 16 row-slots per partition
GCLAMP = 1e-30             # gamma clamp before log (exact-0 gammas)

NS_BF = 6                  # Newton-Schulz iterations in bf16
NS_FP = 2                  # fp32 refinement iterations


def build_nc(ns_bf=NS_BF, ns_fp=NS_FP):
    import concourse.mybir as mybir
    import concourse.tile as tile
    from concourse import bacc
    from concourse.masks import make_identity, make_upper_triangular

    fp32 = mybir.dt.float32
    bf16 = mybir.dt.bfloat16
    AF = mybir.ActivationFunctionType
    OP = mybir.AluOpType
    AX = mybir.AxisListType

    # Bacc (not raw Bass): its compile() pass splits multi-sem sync waits to
    # the 1-wait-per-instruction limit the TRN2 encodings require.
    nc = bacc.Bacc(trn_type="TRN2", target_bir_lowering=False, debug=False)
    keys = nc.dram_tensor("keys", [BPC, T, DK], fp32, kind="ExternalInput").ap()
    values = nc.dram_tensor("values", [BPC, T, DV], fp32, kind="ExternalInput").ap()
    gammas = nc.dram_tensor("gammas", [BPC, T], fp32, kind="ExternalInput").ap()
    queries = nc.dram_tensor("queries", [BPC, NQ, DK], fp32, kind="ExternalInput").ap()
    out = nc.dram_tensor("out", [BPC, NQ, DV], fp32, kind="ExternalOutput").ap()

    with tile.TileContext(nc) as tc:
        const = tc.alloc_tile_pool(name="const", bufs=1)
        gam = tc.alloc_tile_pool(name="gam", bufs=1)
        kp = tc.alloc_tile_pool(name="kp", bufs=2)
        vp = tc.alloc_tile_pool(name="vp", bufs=2)
        vcp = tc.alloc_tile_pool(name="vcp", bufs=2)
        kvbp = tc.alloc_tile_pool(name="kvbp", bufs=2)
        qp = tc.alloc_tile_pool(name="qp", bufs=2)
        qbp = tc.alloc_tile_pool(name="qbp", bufs=1)
        qtp = tc.alloc_tile_pool(name="qtp", bufs=2)
        small = tc.alloc_tile_pool(name="small", bufs=1)
        xs = tc.alloc_tile_pool(name="xs", bufs=2)
        outp = tc.alloc_tile_pool(name="outp", bufs=2)
        ps_as = tc.alloc_tile_pool(name="ps_as", bufs=2, space="PSUM")
        ps_w = tc.alloc_tile_pool(name="ps_w", bufs=5, space="PSUM")

        ident = const.tile([P, P], fp32)
        make_identity(nc, ident)
        ident_bf = const.tile([P, P], bf16)
        make_identity(nc, ident_bf)
        # 4 identity blocks side by side, for group-batched I - A@X residuals
        ident4 = const.tile([P, 4 * P], fp32)
        for i in range(4):
            make_identity(nc, ident4[:, i * P : (i + 1) * P])
        # strict upper triangular (ones above diagonal) and all-ones, for the
        # cross-partition prefix-sum of per-partition gamma-log totals
        utri = const.tile([P, P], fp32)
        make_upper_triangular(nc, utri, val=1.0, diag=False)
        ones2 = const.tile([P, P], fp32)
        nc.gpsimd.memset(ones2[:], 1.0)

        # ---- phase 0: suffix cumprod of gammas (log space) ----
        # g16[p, i, r] = gamma[i, 16p + r]
        g16 = gam.tile([P, BPC, R16], fp32)
        nc.sync.dma_start(
            g16[:], gammas.rearrange("i (p r) -> p i r", r=R16)
        )
        nc.vector.tensor_scalar_max(g16[:], g16[:], GCLAMP)
        nc.scalar.activation(g16[:], g16[:], AF.Ln)
        incl = gam.tile([P, BPC, R16], fp32)
        zz = gam.tile([P, R16], fp32)
        nc.vector.memset(zz[:], 0.0)
        # joiner: make DVE observe the ACT (Ln) dependency before the scans
        joiner = gam.tile([P, 1], fp32)
        nc.vector.tensor_copy(out=joiner[:], in_=g16[:, 0, 0:1])
        for i in range(BPC):
            nc.vector.tensor_tensor_scan(
                incl[:, i, :], g16[:, i, :], zz[:], 0.0, OP.add, OP.add
            )
        # per-partition totals -> cross-partition exclusive prefix + full sum
        ptot = gam.tile([P, BPC], fp32)
        nc.vector.tensor_copy(out=ptot[:], in_=incl[:, :, R16 - 1])
        ps_pre = ps_w.tile([P, 2 * BPC], fp32, tag="w", name="ps_pre")
        nc.tensor.matmul(ps_pre[:, 0:BPC], utri[:], ptot[:])     # offs
        nc.tensor.matmul(ps_pre[:, BPC : 2 * BPC], ones2[:], ptot[:])  # total
        pre_sb = gam.tile([P, 2 * BPC], fp32)
        nc.vector.tensor_copy(out=pre_sb[:], in_=ps_pre[:])
        bias2 = gam.tile([P, BPC], fp32)
        # bias = total - offs  (per partition & memory)
        nc.vector.tensor_tensor(
            bias2[:], pre_sb[:, BPC : 2 * BPC], pre_sb[:, 0:BPC], OP.subtract
        )
        # c_t[p, i, r] = exp(bias - incl) = prod_{s > 16p+r} gamma[i, s]
        c_t = gam.tile([P, BPC, R16], fp32)
        for i in range(BPC):
            nc.scalar.activation(
                c_t[:, i, :], incl[:, i, :], AF.Exp,
                bias=bias2[:, i : i + 1], scale=-1.0,
            )

        # ---- per-memory state tiles ----
        A_sb = [small.tile([P, P], fp32, tag=f"A{i}", name=f"A{i}") for i in range(BPC)]
        A_bf = [small.tile([P, P], bf16, tag=f"Ab{i}", name=f"Ab{i}") for i in range(BPC)]
        ST_sb = [small.tile([P, P], fp32, tag=f"S{i}", name=f"S{i}") for i in range(BPC)]
        Phi_bf = [small.tile([P, P], bf16, tag=f"Pb{i}", name=f"Phib{i}") for i in range(BPC)]
        rs_sb = [small.tile([P, 1], fp32, tag=f"r{i}", name=f"rs{i}") for i in range(BPC)]
        Qb = [qbp.tile([P, R16, DK], bf16, tag=f"qb{i}", name=f"qb{i}") for i in range(BPC)]

        # ---- phase 1: load K/V/Q; bf16 [K | cV]; A = K^T K + I, S^T ----
        for i in range(BPC):
            k_sb = kp.tile([P, R16, DK], fp32, tag="k", name=f"k{i}")
            nc.sync.dma_start(k_sb[:], keys[i].rearrange("(p r) k -> p r k", p=P))
            v_sb = vp.tile([P, R16, DV], fp32, tag="v", name=f"v{i}")
            nc.sync.dma_start(v_sb[:], values[i].rearrange("(p r) k -> p r k", p=P))
            q_sb = qp.tile([P, R16, DK], fp32, tag="q", name=f"q{i}")
            nc.scalar.dma_start(q_sb[:], queries[i].rearrange("(p r) k -> p r k", p=P))

            kvb = kvbp.tile([P, R16, 2 * P], bf16, tag="kvb", name=f"kvb{i}")
            # K cast on ScalarE (cheapest converter); Q cast on GpSimd
            nc.scalar.copy(out=kvb[:, :, 0:DK], in_=k_sb[:])
            nc.gpsimd.tensor_copy(out=Qb[i][:], in_=q_sb[:])
            # V * c fp32 on DVE (fast), then cast to bf16 on ScalarE
            vc = vcp.tile([P, R16, DV], fp32, tag="vc", name=f"vc{i}")
            nc.vector.tensor_tensor(
                vc[:], v_sb[:],
                c_t[:, i, :, None].to_broadcast((P, R16, DV)),
                OP.mult,
            )
            nc.scalar.copy(out=kvb[:, :, DK : 2 * DK], in_=vc[:])

            ps = ps_as.tile([P, 2 * P], fp32, tag="as", name=f"ps_as{i}")
            for r in range(R16):
                nc.tensor.matmul(
                    ps[:], kvb[:, r, 0:DK], kvb[:, r, :],
                    start=(r == 0), stop=(r == R16 - 1),
                )

            # A = K^T K + I ; S^T = K^T diag(c) V ; Jacobi-style NS init
            nc.vector.tensor_tensor(A_sb[i][:], ps[:, 0:P], ident[:], OP.add)
            nc.vector.tensor_copy(out=ST_sb[i][:], in_=ps[:, P : 2 * P])
            nc.scalar.copy(out=A_bf[i][:], in_=A_sb[i][:])
            nc.vector.tensor_reduce(
                rs_sb[i][:], A_sb[i][:], AX.X, OP.add, apply_absolute_value=True
            )
            nc.vector.reciprocal(rs_sb[i][:], rs_sb[i][:])

        # ---- phase 2: Newton-Schulz (residual form), 4 memories per bank ----
        NG = BPC // 4
        Xb = []
        for g in range(NG):
            xw = xs.tile([P, 4 * P], bf16, tag=f"Xb{g}", name=f"Xb{g}_0")
            for i in range(4):
                nc.scalar.activation(
                    xw[:, i * P : (i + 1) * P], ident[:], AF.Copy,
                    scale=rs_sb[4 * g + i][:],
                )
            Xb.append(xw)

        for it in range(ns_bf + ns_fp):
            bf_iter = it < ns_bf
            last_bf = it == ns_bf - 1
            Amat = A_bf if bf_iter else A_sb
            ps_l = []
            for g in range(NG):
                pa = ps_w.tile([P, 4 * P], fp32, tag="w", name=f"pa{it}_{g}")
                for i in range(4):
                    sl = slice(i * P, (i + 1) * P)
                    nc.tensor.matmul(pa[:, sl], Amat[4 * g + i][:], Xb[g][:, sl])
                ps_l.append(pa)
            ex = []
            for g in range(NG):
                eg = xs.tile(
                    [P, 4 * P], bf16 if bf_iter else fp32,
                    tag=f"e{g}_{bf_iter}", name=f"e{it}_{g}",
                )
                nc.vector.scalar_tensor_tensor(
                    eg[:], ps_l[g][:], -1.0, ident4[:], OP.mult, OP.add
                )
                ex.append(eg)
            ps_b = []
            for g in range(NG):
                pb = ps_w.tile([P, 4 * P], fp32, tag="w", name=f"pb{it}_{g}")
                for i in range(4):
                    sl = slice(i * P, (i + 1) * P)
                    nc.tensor.matmul(pb[:, sl], Xb[g][:, sl], ex[g][:, sl])
                ps_b.append(pb)
            for g in range(NG):
                out_fp32 = (not bf_iter) or last_bf
                xn = xs.tile(
                    [P, 4 * P], fp32 if out_fp32 else bf16,
                    tag=f"Xf{g}" if out_fp32 else f"Xb{g}",
                    name=f"X{g}_{it + 1}",
                )
                nc.vector.tensor_tensor(xn[:], Xb[g][:], ps_b[g][:], OP.add)
                Xb[g] = xn

        # ---- phase 3: Phi = R @ S^T (fp32 matmul, bf16 result) ----
        for i in range(BPC):
            g, sl = i // 4, slice((i % 4) * P, (i % 4 + 1) * P)
            ps_phi = ps_w.tile([P, P], fp32, tag="w", name=f"ps_phi{i}")
            nc.tensor.matmul(ps_phi[:], Xb[g][:, sl], ST_sb[i][:])
            nc.scalar.copy(out=Phi_bf[i][:], in_=ps_phi[:])

        # ---- phase 4: out = Q @ Phi (bf16 readout, packed 4 per bank) ----
        for i in range(BPC):
            qt = qtp.tile([P, R16, P], bf16, tag="qt", name=f"qt{i}")
            for r4 in range(R16 // 4):
                ps_qt = ps_w.tile([P, 4 * P], bf16, tag="w", name=f"ps_qt{i}_{r4}")
                for j in range(4):
                    nc.tensor.transpose(
                        ps_qt[:, j * P : (j + 1) * P], Qb[i][:, 4 * r4 + j, :],
                        ident_bf[:],
                    )
                nc.vector.tensor_copy(
                    out=qt[:, 4 * r4 : 4 * r4 + 4, :], in_=ps_qt[:]
                )
            o_sb = outp.tile([P, R16, DV], fp32, tag="o", name=f"o{i}")
            for r4 in range(R16 // 4):
                ps_o = ps_w.tile([P, 4 * P], fp32, tag="w", name=f"ps_o{i}_{r4}")
                for j in range(4):
                    nc.tensor.matmul(
                        ps_o[:, j * P : (j + 1) * P], qt[:, 4 * r4 + j, :],
                        Phi_bf[i][:],
                    )
                nc.scalar.copy(
                    out=o_sb[:, 4 * r4 : 4 * r4 + 4, :], in_=ps_o[:]
                )
            nc.scalar.dma_start(out[i].rearrange("(p r) v -> p r v", p=P), o_sb[:])

        for pool in (ps_w, ps_as, outp, xs, small, qtp, qbp, qp, kvbp, vcp, vp,
                     kp, gam, const):
            pool.release()

    if not nc.is_finalized():
        nc.finalize()
    return nc


def kernel(**inputs) -> np.ndarray:
    keys = np.ascontiguousarray(inputs["keys"], dtype=np.float32)
    values = np.ascontiguousarray(inputs["values"], dtype=np.float32)
    gammas = np.ascontiguousarray(inputs["gammas"], dtype=np.float32)
    queries = np.ascontiguousarray(inputs["queries"], dtype=np.float32)

    from concourse.bass_utils import run_bass_kernel_spmd

    nc = build_nc()
    in_maps = []
    for m in range(NCORES):
        s = slice(m * BPC, (m + 1) * BPC)
        in_maps.append(
            {
                "keys": keys[s],
                "values": values[s],
                "gammas": gammas[s],
                "queries": queries[s],
            }
        )
    res = run_bass_kernel_spmd(nc, in_maps, core_ids=list(range(NCORES)))
    return np.concatenate([res.results[m]["out"] for m in range(NCORES)], axis=0)
